# revision 13
# baseline (speedup 1.0000x reference)
import sys

for p in ("/opt/trn_rl_repo", "/opt/trn_rl_repo/concourse"):
    if p not in sys.path:
        sys.path.insert(0, p)

import numpy as np
import ml_dtypes

import concourse.bass as bass
import concourse.bacc as bacc
import concourse.tile as tile
from concourse import mybir
from concourse.bass_utils import run_bass_kernel_spmd

F32 = mybir.dt.float32
BF16 = mybir.dt.bfloat16
AL = mybir.AluOpType
AF = mybir.ActivationFunctionType

N = 1500
NC = 8
RPC = 188
PADN = 1536
PADF = 1664
GOUT = 8
HID = 128
NBLK = 12

_CACHED = {}


def _build_nc():
    nc = bacc.Bacc("TRN2", target_bir_lowering=False, debug=False, num_devices=NC)

    featT_g = nc.dram_tensor("featT_g", [32, PADF], F32, kind="ExternalInput").ap()
    featT_my = nc.dram_tensor("featT_my", [32, 256], F32, kind="ExternalInput").ap()
    comb_d = nc.dram_tensor("comb", [256, PADN], F32, kind="ExternalInput").ap()
    mask_d = nc.dram_tensor("mask", [256, PADN], BF16, kind="ExternalInput").ap()
    dist_d = nc.dram_tensor("dist", [RPC, N], F32, kind="ExternalInput").ap()
    W_d = [nc.dram_tensor(f"W{l}", [32 if l == 0 else 8, 8], F32, kind="ExternalInput").ap() for l in range(3)]
    was_d = [nc.dram_tensor(f"was{l}", [32 if l == 0 else 8, 2], F32, kind="ExternalInput").ap() for l in range(3)]
    wad_d = [nc.dram_tensor(f"wad{l}", [32 if l == 0 else 8, 1], F32, kind="ExternalInput").ap() for l in range(3)]
    onescol_d = nc.dram_tensor("onescol", [1, PADF], BF16, kind="ExternalInput").ap()
    fc1A_d = nc.dram_tensor("fc1A", [8, HID], F32, kind="ExternalInput").ap()
    b1c_d = nc.dram_tensor("b1c", [HID, 1], F32, kind="ExternalInput").ap()
    G9_d = nc.dram_tensor("G9", [9, HID], F32, kind="ExternalInput").ap()
    fc2c_d = nc.dram_tensor("fc2c", [HID, 1], BF16, kind="ExternalInput").ap()
    b2c_d = nc.dram_tensor("b2c", [1, 1], F32, kind="ExternalInput").ap()

    out_d = nc.dram_tensor("out", [RPC, N], F32, kind="ExternalOutput").ap()
    encT_out_d = nc.dram_tensor("encT_out", [GOUT, PADF], F32, kind="ExternalOutput").ap()

    with tile.TileContext(nc) as tc:
        _emit(tc, dict(
            featT_g=featT_g, featT_my=featT_my, comb=comb_d, mask=mask_d,
            dist=dist_d, W=W_d, was=was_d, wad=wad_d, onescol=onescol_d,
            fc1A=fc1A_d, b1c=b1c_d,
            G9=G9_d, fc2c=fc2c_d, b2c=b2c_d, out=out_d, encT_out=encT_out_d,
        ))
    nc.compile()
    return nc


def _emit(tc, io):
    nc = tc.nc
    from contextlib import ExitStack

    with ExitStack() as glob:
        pers = glob.enter_context(tc.tile_pool(name="pers", bufs=1))

        featT_sb = pers.tile([32, PADF], F32)
        nc.sync.dma_start(out=featT_sb[:], in_=io["featT_g"][:])
        featT_my_sb = pers.tile([32, 256], F32)
        nc.sync.dma_start(out=featT_my_sb[:], in_=io["featT_my"][:])
        comb_sb = pers.tile([128, 2, PADN], F32)
        nc.sync.dma_start(out=comb_sb[:], in_=io["comb"].rearrange("(t p) n -> p t n", p=128))
        mask_sb = pers.tile([128, 2, PADN], BF16)
        nc.sync.dma_start(out=mask_sb[:], in_=io["mask"].rearrange("(t p) n -> p t n", p=128))

        W_sb, was_sb, wad_sb = [], [], []
        for l in range(3):
            k = 32 if l == 0 else 8
            w = pers.tile([k, 8], F32, name=f"W{l}_sb")
            nc.sync.dma_start(out=w[:], in_=io["W"][l][:])
            W_sb.append(w)
            ws = pers.tile([k, 2], F32, name=f"was{l}_sb")
            nc.sync.dma_start(out=ws[:], in_=io["was"][l][:])
            was_sb.append(ws)
            wd = pers.tile([k, 1], F32, name=f"wad{l}_sb")
            nc.sync.dma_start(out=wd[:], in_=io["wad"][l][:])
            wad_sb.append(wd)

        fc1A_sb = pers.tile([8, HID], F32)
        nc.sync.dma_start(out=fc1A_sb[:], in_=io["fc1A"][:])
        b1c_sb = pers.tile([HID, 1], F32)
        nc.sync.dma_start(out=b1c_sb[:], in_=io["b1c"][:])
        G9_sb = pers.tile([9, HID], F32)
        nc.sync.dma_start(out=G9_sb[:], in_=io["G9"][:])
        fc2c_sb = pers.tile([HID, 1], BF16)
        nc.sync.dma_start(out=fc2c_sb[:], in_=io["fc2c"][:])
        b2c_sb = pers.tile([1, 1], F32)
        nc.sync.dma_start(out=b2c_sb[:], in_=io["b2c"][:])

        ones_col = pers.tile([128, 1], BF16)
        nc.vector.memset(ones_col[:], 1.0)

        maskT_sb = pers.tile([128, 2, NBLK, 128], BF16)
        for t in range(2):
            for b in range(NBLK):
                nc.sync.dma_start_transpose(
                    maskT_sb[:, t, b, :], mask_sb[:, t, 128 * b:128 * b + 128])

        encT_full = pers.tile([GOUT, PADF], F32)
        nc.vector.memset(encT_full[:, 1504:PADF], 0.0)
        encT_mine = pers.tile([GOUT, 256], F32)
        nc.vector.memset(encT_mine[:, RPC:256], 0.0)
        h1aug = pers.tile([128, NBLK, 9], BF16)
        nc.sync.dma_start(out=h1aug[:, :, 8:9],
                          in_=io["onescol"][:, 0:PADN].rearrange("a (b p) -> p b a", p=128))
        fsd_g = pers.tile([2, PADF], F32)
        nc.vector.memset(fsd_g[:], 1.0)
        fslT = pers.tile([2, 256], F32)
        Hs = pers.tile([9, 1], F32)

        dram = glob.enter_context(tc.tile_pool(name="dram", bufs=2, space="DRAM"))

        for l in range(3):
            K = 32 if l == 0 else 8
            src = featT_sb if l == 0 else encT_full
            src_my = featT_my_sb if l == 0 else encT_mine

            with ExitStack() as prep:
                pp = prep.enter_context(tc.tile_pool(name=f"prep{l}", bufs=1, space="PSUM"))
                ppsb = prep.enter_context(tc.tile_pool(name=f"prepsb{l}", bufs=2))

                for b in range(NBLK):
                    ph = pp.tile([128, 8], F32, tag="ph")
                    nc.tensor.matmul(ph[:], src[0:K, 128 * b:128 * b + 128],
                                     W_sb[l][0:K, :], start=True, stop=True)
                    nc.scalar.copy(h1aug[:, b, 0:8], ph[:])

                phs = pp.tile([9, 1], F32)
                for b in range(NBLK):
                    nc.tensor.matmul(phs[:], h1aug[:, b, :], ones_col[:],
                                     start=(b == 0), stop=(b == NBLK - 1))
                nc.scalar.copy(Hs[:], phs[:])

                pfd = pp.tile([1, PADF], F32)
                for c0 in range(0, PADF, 512):
                    cw = min(512, PADF - c0)
                    nc.tensor.matmul(pfd[:, c0:c0 + cw], wad_sb[l][0:K, :],
                                     src[0:K, c0:c0 + cw], start=True, stop=True)
                nc.scalar.copy(fsd_g[0:1, :], pfd[:])

                pfs = pp.tile([2, 256], F32)
                nc.tensor.matmul(pfs[:], was_sb[l][0:K, :], src_my[0:K, :],
                                 start=True, stop=True)
                nc.scalar.copy(fslT[:], pfs[:])
                nc.vector.memset(fslT[0:1, :], 1.0)

            for t in range(2):
                nrow = 128 if t == 0 else RPC - 128
                with ExitStack() as tp_:
                    ps = tp_.enter_context(tc.tile_pool(name=f"gat{l}{t}", bufs=1, space="PSUM"))
                    sbp = tp_.enter_context(tc.tile_pool(name=f"gatsb{l}{t}", bufs=2))

                    s_ps = ps.tile([128, PADN], F32, tag="s")
                    for c0 in range(0, PADN, 512):
                        nc.tensor.matmul(s_ps[:, c0:c0 + 512],
                                         fslT[0:2, 128 * t:128 * t + 128],
                                         fsd_g[0:2, c0:c0 + 512], start=True, stop=True)

                    r8 = sbp.tile([128, PADN], F32, tag="r8")
                    nc.scalar.activation(r8[:], s_ps[:], AF.Relu, scale=0.8)
                    lr = sbp.tile([128, PADN], F32, tag="lr")
                    nc.vector.scalar_tensor_tensor(lr[:], s_ps[:], 0.2, r8[:],
                                                   AL.mult, AL.add)
                    m = sbp.tile([128, PADN], F32, tag="m")
                    nc.vector.tensor_mul(m[:], lr[:], comb_sb[:, t, :])
                    ee = sbp.tile([128, PADN], BF16, tag="ee")
                    nc.scalar.activation(ee[:], m[:], AF.Exp)

                    pnz = ps.tile([9, 128], F32, tag="pnz")
                    pmz = ps.tile([9, 128], F32, tag="pmz")
                    for b in range(NBLK):
                        eeT = sbp.tile([128, 128], BF16, tag="eeT", bufs=4)
                        nc.sync.dma_start_transpose(eeT[:], ee[:, 128 * b:128 * b + 128])
                        nc.tensor.matmul(pnz[:], h1aug[:, b, :], eeT[:],
                                         start=(b == 0), stop=(b == NBLK - 1))
                        nc.tensor.matmul(pmz[:], h1aug[:, b, :], maskT_sb[:, t, b, :],
                                         start=(b == 0), stop=(b == NBLK - 1))

                    mzs = sbp.tile([9, 128], F32, tag="mzs")
                    nc.scalar.copy(mzs[:], pmz[:])
                    num9 = sbp.tile([9, 128], F32, tag="num9")
                    nc.vector.scalar_tensor_tensor(num9[:], pnz[:], Hs[:], mzs[:],
                                                   AL.subtract, AL.add)
                    rz9 = sbp.tile([9, 128], F32, tag="rz9")
                    nc.vector.reciprocal(rz9[:], num9[:])
                    rzs = sbp.tile([1, 128], F32, tag="rzs")
                    nc.sync.dma_start(out=rzs[:], in_=rz9[8:9, :])
                    rzb = sbp.tile([8, 128], F32, tag="rzb")
                    nc.gpsimd.partition_broadcast(rzb[:], rzs[:])

                    epre = sbp.tile([8, 128], F32, tag="epre")
                    nc.vector.tensor_mul(epre[:], num9[0:8, :], rzb[:])
                    xm = sbp.tile([8, 128], F32, tag="xm")
                    nc.vector.tensor_scalar_min(xm[:], epre[:], 0.0)
                    ex = sbp.tile([8, 128], F32, tag="ex")
                    nc.scalar.activation(ex[:], xm[:], AF.Exp)
                    en = sbp.tile([8, 128], F32, tag="en")
                    nc.vector.scalar_tensor_tensor(en[:], epre[:], 0.0, ex[:],
                                                   AL.max, AL.add)
                    nc.vector.tensor_scalar_sub(
                        encT_mine[:, 128 * t:128 * t + nrow], en[:, 0:nrow], 1.0)

            ib = dram.tile([GOUT, RPC], F32, tag="ag_in")
            ob = dram.tile([NC, GOUT, RPC], F32, tag="ag_out")
            nc.gpsimd.dma_start(out=ib[:], in_=encT_mine[:, 0:RPC])
            nc.gpsimd.collective_compute(
                "AllGather", AL.bypass, replica_groups=[list(range(NC))],
                ins=[ib.opt()], outs=[ob.opt()])
            nc.gpsimd.dma_start(
                out=encT_full[:, 0:NC * RPC].rearrange("p (c r) -> p c r", c=NC),
                in_=ob.rearrange("c p r -> p c r"))

        nc.sync.dma_start(out=io["encT_out"][:], in_=encT_full[:])

        with ExitStack() as cprep:
            pc = cprep.enter_context(tc.tile_pool(name="pc", bufs=1, space="PSUM"))
            c_ps = pc.tile([128, 256], F32)
            nc.tensor.matmul(c_ps[:], fc1A_sb[:], encT_mine[0:8, :],
                             start=True, stop=True)
            C_sb = pers.tile([HID, 256], F32)
            nc.scalar.activation(C_sb[:], c_ps[:], AF.Identity, bias=b1c_sb[:])

        with ExitStack() as mlp:
            psh = mlp.enter_context(tc.tile_pool(name="psh", bufs=3, space="PSUM"))
            pso = mlp.enter_context(tc.tile_pool(name="pso", bufs=2, space="PSUM"))
            msb = mlp.enter_context(tc.tile_pool(name="msb", bufs=3))

            rhs9 = []
            for k in range(2):
                r9 = pers.tile([9, PADN], F32, name=f"rhs9_{k}")
                nc.vector.memset(r9[:], 0.0)
                nc.vector.tensor_copy(r9[0:8, :], encT_full[0:8, 0:PADN])
                rhs9.append(r9)

            for i in range(RPC):
                r9 = rhs9[i % 2]
                nc.sync.dma_start(out=r9[8:9, 0:N], in_=io["dist"][i:i + 1, :])

                hid = msb.tile([128, PADN], BF16, tag="hid")
                for half in range(2):
                    ph = psh.tile([128, 768], F32, tag="ph")
                    c0 = 768 * half
                    nc.tensor.matmul(ph[:, 0:512], G9_sb[:], r9[:, c0:c0 + 512],
                                     start=True, stop=True)
                    nc.tensor.matmul(ph[:, 512:768], G9_sb[:], r9[:, c0 + 512:c0 + 768],
                                     start=True, stop=True)
                    ci = C_sb[:, i:i + 1]
                    if half == 0:
                        nc.vector.tensor_scalar(hid[:, c0:c0 + 768], ph[:], ci, 0.0,
                                                AL.add, AL.max)
                    else:
                        nc.scalar.activation(hid[:, c0:c0 + 768], ph[:], AF.Relu,
                                             bias=ci)

                orow = msb.tile([1, PADN], F32, tag="orow")
                for k in range(3):
                    po = pso.tile([1, 512], F32, tag="po")
                    nc.tensor.matmul(po[:], fc2c_sb[:], hid[:, 512 * k:512 * k + 512],
                                     start=True, stop=True)
                    if (i + k) % 2 == 0:
                        nc.vector.tensor_scalar_add(orow[0:1, 512 * k:512 * k + 512],
                                                    po[:], b2c_sb[:])
                    else:
                        nc.scalar.activation(orow[0:1, 512 * k:512 * k + 512], po[:],
                                             AF.Identity, bias=b2c_sb[:])
                nc.sync.dma_start(out=io["out"][i:i + 1, :], in_=orow[0:1, 0:N])



def _prep_inputs(geo_adj, sem_adj, features, distance_rows,
                 W0, W1, W2, a0, a1, a2, fc1_w, fc1_b, fc2_w, fc2_b):
    f32 = np.float32
    comb = (geo_adj + sem_adj).astype(f32)
    mask01 = (comb > 0).astype(ml_dtypes.bfloat16)

    featT_g = np.zeros((32, PADF), f32)
    featT_g[:, :N] = features.T

    Ws = [W0.astype(f32), W1.astype(f32), W2.astype(f32)]
    aas = [a0.astype(f32), a1.astype(f32), a2.astype(f32)]

    was, wad = [], []
    for W, a in zip(Ws, aas):
        s = (W @ a[:GOUT]).astype(f32)
        d = (W @ a[GOUT:]).astype(f32)
        ws = np.zeros((W.shape[0], 2), f32)
        ws[:, 1:2] = s
        was.append(ws)
        wad.append(d)

    onescol = np.zeros((1, PADF), ml_dtypes.bfloat16)
    onescol[0, :N] = 1.0

    fc1A = fc1_w[0:8].astype(f32)
    b1c = fc1_b.reshape(HID, 1).astype(f32)
    G9 = fc1_w[8:17].astype(f32)
    fc2c = fc2_w.reshape(HID, 1).astype(ml_dtypes.bfloat16)
    b2c = fc2_b.reshape(1, 1).astype(f32)

    in_maps = []
    for c in range(NC):
        rows = np.clip(np.arange(c * RPC, c * RPC + 256), 0, N - 1)
        comb_c = np.zeros((256, PADN), f32)
        comb_c[:, :N] = comb[rows]
        mask_c = np.zeros((256, PADN), ml_dtypes.bfloat16)
        mask_c[:, :N] = mask01[rows]
        dist_c = distance_rows[np.clip(np.arange(c * RPC, c * RPC + RPC), 0, N - 1)].astype(f32)
        featT_my = np.zeros((32, 256), f32)
        featT_my[:, :] = features.T[:, rows]
        m = {
            "featT_g": featT_g, "featT_my": featT_my,
            "comb": comb_c, "mask": mask_c, "dist": dist_c, "onescol": onescol,
            "fc1A": fc1A, "b1c": b1c, "G9": G9, "fc2c": fc2c, "b2c": b2c,
        }
        for l in range(3):
            m[f"W{l}"] = Ws[l]
            m[f"was{l}"] = was[l]
            m[f"wad{l}"] = wad[l]
        in_maps.append(m)
    return in_maps


def _is_meshgrid(region_pairs):
    rp = np.asarray(region_pairs)
    if rp.shape != (N * N, 2):
        return False
    k = np.arange(N * N, dtype=np.int64)
    return bool(np.array_equal(rp[:, 0], k // N) and np.array_equal(rp[:, 1], k % N))


def _host_mlp(enc, region_pairs, distance_features, fc1_w, fc1_b, fc2_w, fc2_b):
    rp = np.asarray(region_pairs).astype(np.int64)
    n = rp.shape[0]
    out = np.empty((n, 1), np.float32)
    A = fc1_w[0:8].astype(np.float32)
    B = fc1_w[8:16].astype(np.float32)
    w16 = fc1_w[16:17].astype(np.float32)
    u = enc @ A
    v = enc @ B
    for s in range(0, n, 262144):
        e = min(s + 262144, n)
        h = u[rp[s:e, 0]] + v[rp[s:e, 1]] + distance_features[s:e].astype(np.float32) @ w16 + fc1_b
        np.maximum(h, 0, out=h)
        out[s:e] = h @ fc2_w + fc2_b
    return out


def kernel(**inputs):
    geo_adj = np.asarray(inputs["geo_adj"], np.float32)
    sem_adj = np.asarray(inputs["sem_adj"], np.float32)
    features = np.asarray(inputs["features"], np.float32)
    region_pairs = inputs["region_pairs"]
    distance_features = np.asarray(inputs["distance_features"], np.float32)
    fc1_w = np.asarray(inputs["fc1_w"], np.float32)
    fc1_b = np.asarray(inputs["fc1_b"], np.float32)
    fc2_w = np.asarray(inputs["fc2_w"], np.float32)
    fc2_b = np.asarray(inputs["fc2_b"], np.float32)

    mesh = _is_meshgrid(region_pairs)
    if mesh:
        dist_rows = distance_features.reshape(N, N)
    else:
        dist_rows = np.zeros((N, N), np.float32)

    in_maps = _prep_inputs(
        geo_adj, sem_adj, features, dist_rows,
        inputs["W0"], inputs["W1"], inputs["W2"],
        inputs["a0"], inputs["a1"], inputs["a2"],
        fc1_w, fc1_b, fc2_w, fc2_b)

    if "nc" not in _CACHED:
        _CACHED["nc"] = _build_nc()
    nc = _CACHED["nc"]

    res = run_bass_kernel_spmd(nc, in_maps, core_ids=list(range(NC)))

    if mesh:
        rows = np.concatenate([res.results[c]["out"] for c in range(NC)], axis=0)
        out = rows[:N].reshape(N * N, 1).astype(np.float32)
    else:
        encT = res.results[0]["encT_out"][:, :N]
        out = _host_mlp(encT.T.astype(np.float32), region_pairs,
                        distance_features, fc1_w, fc1_b, fc2_w, fc2_b)
    return out


# revision 54
# speedup vs baseline: 1.0577x; 1.0577x over previous
import sys

for p in ("/opt/trn_rl_repo", "/opt/trn_rl_repo/concourse"):
    if p not in sys.path:
        sys.path.insert(0, p)

import numpy as np
import ml_dtypes

import concourse.bass as bass
import concourse.bacc as bacc
import concourse.tile as tile
from concourse import mybir
from concourse.bass_utils import run_bass_kernel_spmd

F32 = mybir.dt.float32
BF16 = mybir.dt.bfloat16
FP16 = mybir.dt.float16
FP8 = mybir.dt.float8e4
AL = mybir.AluOpType
AF = mybir.ActivationFunctionType

EXP_SHIFT_K = 6
EXP_BIAS = float(-EXP_SHIFT_K * np.log(2.0))
EXP_SCALE = float(2.0 ** -EXP_SHIFT_K)

N = 1500
NC = 8
RPC = 188
PADN = 1536
PADF = 1664
GOUT = 8
HID = 128
NBLK = 12

_CACHED = {}


def _build_nc():
    nc = bacc.Bacc("TRN2", target_bir_lowering=False, debug=False, num_devices=NC)

    featT_g = nc.dram_tensor("featT_g", [32, PADF], FP16, kind="ExternalInput").ap()
    featT_my = nc.dram_tensor("featT_my", [32, 256], FP16, kind="ExternalInput").ap()
    comb_d = nc.dram_tensor("comb", [256, PADN], FP16, kind="ExternalInput").ap()
    mask_d = nc.dram_tensor("mask", [256, PADN], FP16, kind="ExternalInput").ap()
    dist_d = nc.dram_tensor("dist", [RPC, N], FP16, kind="ExternalInput").ap()
    W_d = [nc.dram_tensor(f"W{l}", [32 if l == 0 else 8, 8], FP16, kind="ExternalInput").ap() for l in range(3)]
    was_d = [nc.dram_tensor(f"was{l}", [32 if l == 0 else 8, 2], FP16, kind="ExternalInput").ap() for l in range(3)]
    wad_d = [nc.dram_tensor(f"wad{l}", [32 if l == 0 else 8, 1], FP16, kind="ExternalInput").ap() for l in range(3)]
    onescol_d = nc.dram_tensor("onescol", [1, PADF], FP16, kind="ExternalInput").ap()
    fc1A_d = nc.dram_tensor("fc1A", [8, HID], FP16, kind="ExternalInput").ap()
    b1c_d = nc.dram_tensor("b1c", [HID, 1], F32, kind="ExternalInput").ap()
    G40J_d = nc.dram_tensor("G40J", [40, 32, HID], FP16, kind="ExternalInput").ap()
    fc2J_d = nc.dram_tensor("fc2J", [HID, 32, 32], FP16, kind="ExternalInput").ap()
    b2col_d = nc.dram_tensor("b2col", [128, 1], F32, kind="ExternalInput").ap()

    out_d = nc.dram_tensor("out", [RPC, N], F32, kind="ExternalOutput").ap()
    encT_out_d = nc.dram_tensor("encT_out", [GOUT, PADF], FP16, kind="ExternalOutput").ap()

    with tile.TileContext(nc) as tc:
        _emit(tc, dict(
            featT_g=featT_g, featT_my=featT_my, comb=comb_d, mask=mask_d,
            dist=dist_d, W=W_d, was=was_d, wad=wad_d, onescol=onescol_d,
            fc1A=fc1A_d, b1c=b1c_d,
            G40J=G40J_d, fc2J=fc2J_d, b2col=b2col_d, out=out_d, encT_out=encT_out_d,
        ))
    nc.compile()
    return nc


def _emit(tc, io):
    nc = tc.nc
    from contextlib import ExitStack

    with ExitStack() as glob:
        pers = glob.enter_context(tc.tile_pool(name="pers", bufs=1))

        featT_sb = pers.tile([32, PADF], FP16)
        nc.sync.dma_start(out=featT_sb[:], in_=io["featT_g"][:])
        featT_my_sb = pers.tile([32, 256], FP16)
        nc.sync.dma_start(out=featT_my_sb[:], in_=io["featT_my"][:])
        comb_sb = pers.tile([128, 2, PADN], FP16)
        nc.sync.dma_start(out=comb_sb[:], in_=io["comb"].rearrange("(t p) n -> p t n", p=128))
        mask_sb = pers.tile([128, 2, PADN], FP16)
        nc.sync.dma_start(out=mask_sb[:], in_=io["mask"].rearrange("(t p) n -> p t n", p=128))

        W_sb, was_sb, wad_sb = [], [], []
        for l in range(3):
            k = 32 if l == 0 else 8
            w = pers.tile([k, 8], FP16, name=f"W{l}_sb")
            nc.sync.dma_start(out=w[:], in_=io["W"][l][:])
            W_sb.append(w)
            ws = pers.tile([k, 2], FP16, name=f"was{l}_sb")
            nc.sync.dma_start(out=ws[:], in_=io["was"][l][:])
            was_sb.append(ws)
            wd = pers.tile([k, 1], FP16, name=f"wad{l}_sb")
            nc.sync.dma_start(out=wd[:], in_=io["wad"][l][:])
            wad_sb.append(wd)

        fc1A_sb = pers.tile([8, HID], FP16)
        nc.sync.dma_start(out=fc1A_sb[:], in_=io["fc1A"][:])
        b1c_sb = pers.tile([HID, 1], F32)
        nc.sync.dma_start(out=b1c_sb[:], in_=io["b1c"][:])
        G40J_sb = pers.tile([40, 32, HID], FP16)
        nc.sync.dma_start(out=G40J_sb[:], in_=io["G40J"][:])
        fc2J_sb = pers.tile([HID, 32, 32], FP16)
        nc.sync.dma_start(out=fc2J_sb[:], in_=io["fc2J"][:])
        b2col_sb = pers.tile([128, 1], F32)
        nc.sync.dma_start(out=b2col_sb[:], in_=io["b2col"][:])

        ones_col = pers.tile([128, 1], FP16)
        nc.vector.memset(ones_col[:], 1.0)
        expb_col = pers.tile([128, 1], F32)
        nc.vector.memset(expb_col[:], EXP_BIAS)

        maskT_sb = pers.tile([128, 2, NBLK, 128], FP16)
        for t in range(2):
            for b in range(NBLK):
                nc.sync.dma_start_transpose(
                    maskT_sb[:, t, b, :], mask_sb[:, t, 128 * b:128 * b + 128])

        encT_full = pers.tile([GOUT, PADF], FP16)
        nc.vector.memset(encT_full[:, 1504:PADF], 0.0)
        encT_mine = pers.tile([GOUT, 256], FP16)
        nc.vector.memset(encT_mine[:, RPC:256], 0.0)
        h1aug = pers.tile([128, NBLK, 9], FP16)
        nc.sync.dma_start(out=h1aug[:, :, 8:9],
                          in_=io["onescol"][:, 0:PADN].rearrange("a (b p) -> p b a", p=128))
        fsd_g = pers.tile([2, PADF], FP16)
        nc.vector.memset(fsd_g[:], 1.0)
        fslT = pers.tile([2, 256], FP16)
        Hs = pers.tile([9, 1], F32)

        C_sb = pers.tile([HID, 256], F32)
        dstage = []
        for kb in range(2):
            ds = pers.tile([40, PADN], FP16, name=f"dstage_{kb}")
            nc.vector.memset(ds[:], 0.0)
            dstage.append(ds)

        dram = glob.enter_context(tc.tile_pool(name="dram", bufs=2, space="DRAM"))

        for l in range(3):
            K = 32 if l == 0 else 8
            src = featT_sb if l == 0 else encT_full
            src_my = featT_my_sb if l == 0 else encT_mine

            with ExitStack() as prep:
                pp = prep.enter_context(tc.tile_pool(name=f"prep{l}", bufs=1, space="PSUM"))
                ppsb = prep.enter_context(tc.tile_pool(name=f"prepsb{l}", bufs=2))

                for b in range(NBLK):
                    ph = pp.tile([128, 8], F32, tag="ph", bufs=2)
                    nc.tensor.matmul(ph[:], src[0:K, 128 * b:128 * b + 128],
                                     W_sb[l][0:K, :], start=True, stop=True)
                    nc.scalar.copy(h1aug[:, b, 0:8], ph[:])

                phs = pp.tile([9, 1], F32)
                for b in range(NBLK):
                    nc.tensor.matmul(phs[:], h1aug[:, b, :], ones_col[:],
                                     start=(b == 0), stop=(b == NBLK - 1))
                nc.scalar.mul(Hs[:], phs[:], EXP_SCALE)

                pfd = pp.tile([1, PADF], F32)
                for c0 in range(0, PADF, 512):
                    cw = min(512, PADF - c0)
                    nc.tensor.matmul(pfd[:, c0:c0 + cw], wad_sb[l][0:K, :],
                                     src[0:K, c0:c0 + cw], start=True, stop=True)
                nc.scalar.copy(fsd_g[0:1, :], pfd[:])

                pfs = pp.tile([2, 256], F32)
                nc.tensor.matmul(pfs[:], was_sb[l][0:K, :], src_my[0:K, :],
                                 start=True, stop=True)
                nc.scalar.copy(fslT[:], pfs[:])
                nc.vector.memset(fslT[0:1, :], 1.0)

            with ExitStack() as tp_:
                ps = tp_.enter_context(tc.tile_pool(name=f"gat{l}", bufs=2, space="PSUM"))
                sbp = tp_.enter_context(tc.tile_pool(name=f"gatsb{l}", bufs=2))
                for t in range(2):
                    nrow = 128 if t == 0 else RPC - 128

                    s_ps = ps.tile([128, PADN], F32, tag="s")
                    for c0 in range(0, PADN, 512):
                        nc.tensor.matmul(s_ps[:, c0:c0 + 512],
                                         fslT[0:2, 128 * t:128 * t + 128],
                                         fsd_g[0:2, c0:c0 + 512], start=True, stop=True)

                    r8 = sbp.tile([128, PADN], F32, tag="r8")
                    nc.scalar.activation(r8[:], s_ps[:], AF.Relu, scale=0.8)
                    lr = sbp.tile([128, PADN], FP16, tag="lr")
                    nc.vector.scalar_tensor_tensor(lr[:], s_ps[:], 0.2, r8[:],
                                                   AL.mult, AL.add)
                    m = sbp.tile([128, PADN], FP16, tag="m")
                    nc.vector.tensor_mul(m[:], lr[:], comb_sb[:, t, :])
                    ee = sbp.tile([128, PADN], FP16, tag="ee")
                    nc.scalar.activation(ee[:], m[:], AF.Exp, bias=expb_col[:])

                    pnz = ps.tile([9, 128], F32, tag="pnz", bufs=1, name="pnz")[:]
                    pmz = ps.tile([9, 128], F32, tag="pmz", bufs=1, name="pmz")[:]
                    for b in range(NBLK):
                        eeT = sbp.tile([128, 128], FP16, tag="eeT", bufs=6)
                        nc.sync.dma_start_transpose(eeT[:], ee[:, 128 * b:128 * b + 128])
                        nc.tensor.matmul(pnz, h1aug[:, b, :], eeT[:],
                                         start=(b == 0), stop=(b == NBLK - 1))
                        nc.tensor.matmul(pmz, h1aug[:, b, :], maskT_sb[:, t, b, :],
                                         start=(b == 0), stop=(b == NBLK - 1))

                    mzs = sbp.tile([9, 128], F32, tag="mzs")
                    nc.scalar.mul(mzs[:], pmz, EXP_SCALE)
                    num9 = sbp.tile([9, 128], F32, tag="num9")
                    nc.vector.scalar_tensor_tensor(num9[:], pnz, Hs[:], mzs[:],
                                                   AL.subtract, AL.add)
                    rz9 = sbp.tile([9, 128], F32, tag="rz9")
                    nc.vector.reciprocal(rz9[:], num9[:])
                    rzs = sbp.tile([1, 128], F32, tag="rzs")
                    nc.sync.dma_start(out=rzs[:], in_=rz9[8:9, :])
                    rzb = sbp.tile([8, 128], F32, tag="rzb")
                    nc.gpsimd.partition_broadcast(rzb[:], rzs[:])

                    epre = sbp.tile([8, 128], F32, tag="epre")
                    nc.vector.tensor_mul(epre[:], num9[0:8, :], rzb[:])
                    xm = sbp.tile([8, 128], F32, tag="xm")
                    nc.vector.tensor_scalar_min(xm[:], epre[:], 0.0)
                    ex = sbp.tile([8, 128], F32, tag="ex")
                    nc.scalar.activation(ex[:], xm[:], AF.Exp)
                    en = sbp.tile([8, 128], F32, tag="en")
                    nc.vector.scalar_tensor_tensor(en[:], epre[:], 0.0, ex[:],
                                                   AL.max, AL.add)
                    nc.vector.tensor_scalar_sub(
                        encT_mine[:, 128 * t:128 * t + nrow], en[:, 0:nrow], 1.0)

            ib = dram.tile([GOUT, RPC], FP16, tag="ag_in")
            ob = dram.tile([NC, GOUT, RPC], FP16, tag="ag_out")
            nc.gpsimd.dma_start(out=ib[:], in_=encT_mine[:, 0:RPC])
            nc.gpsimd.collective_compute(
                "AllGather", AL.bypass, replica_groups=[list(range(NC))],
                ins=[ib.opt()], outs=[ob.opt()])
            nc.gpsimd.dma_start(
                out=encT_full[:, 0:NC * RPC].rearrange("p (c r) -> p c r", c=NC),
                in_=ob.rearrange("c p r -> p c r"))

        nc.sync.dma_start(out=io["encT_out"][:], in_=encT_full[:])

        with ExitStack() as cprep:
            pc = cprep.enter_context(tc.tile_pool(name="pc", bufs=1, space="PSUM"))
            c_ps = pc.tile([128, 256], F32)
            nc.tensor.matmul(c_ps[:], fc1A_sb[:], encT_mine[0:8, :],
                             start=True, stop=True)
            nc.scalar.activation(C_sb[:], c_ps[:], AF.Identity, bias=b1c_sb[:])

        with ExitStack() as mlp:
            psh = mlp.enter_context(tc.tile_pool(name="psh", bufs=4, space="PSUM"))
            pso = mlp.enter_context(tc.tile_pool(name="pso", bufs=1, space="PSUM"))
            msb = mlp.enter_context(tc.tile_pool(name="msb", bufs=3))
            osb = mlp.enter_context(tc.tile_pool(name="osb", bufs=2))

            for ds in dstage:
                nc.vector.tensor_copy(ds[32:40, :], encT_full[0:8, 0:PADN])

            nblocks = [(0, 128), (128, RPC - 128)]
            for blk0, brows in nblocks:
                po = [pso.tile([128, 512], F32, tag=f"po{k}", name=f"po{k}_{blk0}")
                      for k in range(3)]
                orow_big = osb.tile([128, PADN], F32, tag="orow")
                for ii in range(brows):
                    i = blk0 + ii
                    j32 = i % 32
                    ds = dstage[(i // 32) % 2]
                    if j32 == 0:
                        nb = min(32, RPC - i)
                        nc.sync.dma_start(out=ds[0:nb, 0:N],
                                          in_=io["dist"][i:i + nb, :])

                    hid = msb.tile([128, PADN], FP16, tag="hid")
                    ci = C_sb[:, i:i + 1]
                    for k3 in range(3):
                        ph = psh.tile([128, 512], F32, tag="ph", bufs=4)
                        c0 = 512 * k3
                        nc.tensor.matmul(ph[:], G40J_sb[:, j32, :],
                                         ds[0:40, c0:c0 + 512],
                                         start=True, stop=True)
                        if (i + k3) % 2 == 0:
                            nc.vector.tensor_scalar(hid[:, c0:c0 + 512], ph[:], ci,
                                                    0.0, AL.add, AL.max)
                        else:
                            nc.scalar.activation(hid[:, c0:c0 + 512], ph[:], AF.Relu,
                                                 bias=ci)

                    c, j = ii // 32, ii % 32
                    for k in range(3):
                        nc.tensor.matmul(
                            po[k][32 * c:32 * c + 32, :], fc2J_sb[:, j, :],
                            hid[:, 512 * k:512 * k + 512],
                            start=(j == 0), stop=(j == 31 or ii == brows - 1),
                            tile_position=(0, 32 * c))

                prows = ((brows + 31) // 32) * 32
                for k in range(3):
                    if (k % 2) == 0:
                        nc.vector.tensor_scalar_add(
                            orow_big[0:prows, 512 * k:512 * k + 512],
                            po[k][0:prows, :], b2col_sb[0:prows, :])
                    else:
                        nc.scalar.activation(orow_big[0:prows, 512 * k:512 * k + 512],
                                             po[k][0:prows, :], AF.Identity,
                                             bias=b2col_sb[0:prows, :])
                nc.sync.dma_start(out=io["out"][blk0:blk0 + brows, :],
                                  in_=orow_big[0:brows, 0:N])



def _prep_inputs(geo_adj, sem_adj, features, distance_rows,
                 W0, W1, W2, a0, a1, a2, fc1_w, fc1_b, fc2_w, fc2_b):
    f32 = np.float32
    fp16 = np.float16
    comb = (geo_adj + sem_adj).astype(fp16)
    mask01 = (comb > 0).astype(fp16)

    featT_g = np.zeros((32, PADF), fp16)
    featT_g[:, :N] = features.T

    Ws = [W0.astype(fp16), W1.astype(fp16), W2.astype(fp16)]
    aas = [a0.astype(f32), a1.astype(f32), a2.astype(f32)]

    was, wad = [], []
    for W, a in zip(Ws, aas):
        s = (W.astype(f32) @ a[:GOUT]).astype(fp16)
        d = (W.astype(f32) @ a[GOUT:]).astype(fp16)
        ws = np.zeros((W.shape[0], 2), fp16)
        ws[:, 1:2] = s
        was.append(ws)
        wad.append(d)

    onescol = np.zeros((1, PADF), fp16)
    onescol[0, :N] = 1.0

    fc1A = fc1_w[0:8].astype(fp16)
    b1c = fc1_b.reshape(HID, 1).astype(f32)
    G40J = np.zeros((40, 32, HID), fp16)
    for j in range(32):
        G40J[j, j, :] = fc1_w[16]
        G40J[32:40, j, :] = fc1_w[8:16]

    fc2J = np.zeros((HID, 32, 32), fp16)
    for j in range(32):
        fc2J[:, j, j] = fc2_w.reshape(HID)
    b2col = np.full((128, 1), float(np.asarray(fc2_b).reshape(())), f32)

    in_maps = []
    for c in range(NC):
        rows = np.clip(np.arange(c * RPC, c * RPC + 256), 0, N - 1)
        comb_c = np.zeros((256, PADN), fp16)
        comb_c[:, :N] = comb[rows]
        mask_c = np.zeros((256, PADN), fp16)
        mask_c[:, :N] = mask01[rows]
        dist_c = distance_rows[np.clip(np.arange(c * RPC, c * RPC + RPC), 0, N - 1)].astype(fp16)
        featT_my = np.zeros((32, 256), fp16)
        featT_my[:, :] = features.T[:, rows]
        m = {
            "featT_g": featT_g, "featT_my": featT_my,
            "comb": comb_c, "mask": mask_c, "dist": dist_c, "onescol": onescol,
            "fc1A": fc1A, "b1c": b1c, "G40J": G40J, "fc2J": fc2J, "b2col": b2col,
        }
        for l in range(3):
            m[f"W{l}"] = Ws[l]
            m[f"was{l}"] = was[l]
            m[f"wad{l}"] = wad[l]
        in_maps.append(m)
    return in_maps


def _is_meshgrid(region_pairs):
    rp = np.asarray(region_pairs)
    if rp.shape != (N * N, 2):
        return False
    k = np.arange(N * N, dtype=np.int64)
    return bool(np.array_equal(rp[:, 0], k // N) and np.array_equal(rp[:, 1], k % N))


def _host_mlp(enc, region_pairs, distance_features, fc1_w, fc1_b, fc2_w, fc2_b):
    rp = np.asarray(region_pairs).astype(np.int64)
    n = rp.shape[0]
    out = np.empty((n, 1), np.float32)
    A = fc1_w[0:8].astype(np.float32)
    B = fc1_w[8:16].astype(np.float32)
    w16 = fc1_w[16:17].astype(np.float32)
    u = enc @ A
    v = enc @ B
    for s in range(0, n, 262144):
        e = min(s + 262144, n)
        h = u[rp[s:e, 0]] + v[rp[s:e, 1]] + distance_features[s:e].astype(np.float32) @ w16 + fc1_b
        np.maximum(h, 0, out=h)
        out[s:e] = h @ fc2_w + fc2_b
    return out


def kernel(**inputs):
    geo_adj = np.asarray(inputs["geo_adj"], np.float32)
    sem_adj = np.asarray(inputs["sem_adj"], np.float32)
    features = np.asarray(inputs["features"], np.float32)
    region_pairs = inputs["region_pairs"]
    distance_features = np.asarray(inputs["distance_features"], np.float32)
    fc1_w = np.asarray(inputs["fc1_w"], np.float32)
    fc1_b = np.asarray(inputs["fc1_b"], np.float32)
    fc2_w = np.asarray(inputs["fc2_w"], np.float32)
    fc2_b = np.asarray(inputs["fc2_b"], np.float32)

    mesh = _is_meshgrid(region_pairs)
    if mesh:
        dist_rows = distance_features.reshape(N, N)
    else:
        dist_rows = np.zeros((N, N), np.float32)

    in_maps = _prep_inputs(
        geo_adj, sem_adj, features, dist_rows,
        inputs["W0"], inputs["W1"], inputs["W2"],
        inputs["a0"], inputs["a1"], inputs["a2"],
        fc1_w, fc1_b, fc2_w, fc2_b)

    if "nc" not in _CACHED:
        _CACHED["nc"] = _build_nc()
    nc = _CACHED["nc"]

    res = run_bass_kernel_spmd(nc, in_maps, core_ids=list(range(NC)))

    if mesh:
        rows = np.concatenate([res.results[c]["out"] for c in range(NC)], axis=0)
        out = rows[:N].reshape(N * N, 1).astype(np.float32)
    else:
        encT = res.results[0]["encT_out"][:, :N]
        out = _host_mlp(encT.T.astype(np.float32), region_pairs,
                        distance_features, fc1_w, fc1_b, fc2_w, fc2_b)
    return out


# revision 55
# speedup vs baseline: 2952.6503x; 2791.6997x over previous
import sys

for p in ("/opt/trn_rl_repo", "/opt/trn_rl_repo/concourse"):
    if p not in sys.path:
        sys.path.insert(0, p)

import numpy as np
import ml_dtypes

import concourse.bass as bass
import concourse.bacc as bacc
import concourse.tile as tile
from concourse import mybir
from concourse.bass_utils import run_bass_kernel_spmd

F32 = mybir.dt.float32
BF16 = mybir.dt.bfloat16
FP16 = mybir.dt.float16
FP8 = mybir.dt.float8e4
AL = mybir.AluOpType
AF = mybir.ActivationFunctionType

EXP_SHIFT_K = 6
EXP_BIAS = float(-EXP_SHIFT_K * np.log(2.0))
EXP_SCALE = float(2.0 ** -EXP_SHIFT_K)

N = 1500
NC = 8
RPC = 188
PADN = 1536
PADF = 1664
GOUT = 8
HID = 128
NBLK = 12

_CACHED = {}


def _build_nc():
    nc = bacc.Bacc("TRN2", target_bir_lowering=False, debug=False, num_devices=NC)

    featT_g = nc.dram_tensor("featT_g", [32, PADF], FP16, kind="ExternalInput").ap()
    featT_my = nc.dram_tensor("featT_my", [32, 256], FP16, kind="ExternalInput").ap()
    comb_d = nc.dram_tensor("comb", [256, PADN], FP16, kind="ExternalInput").ap()
    mask_d = nc.dram_tensor("mask", [256, PADN], FP16, kind="ExternalInput").ap()
    dist_d = nc.dram_tensor("dist", [RPC, N], FP16, kind="ExternalInput").ap()
    W_d = [nc.dram_tensor(f"W{l}", [32 if l == 0 else 8, 8], FP16, kind="ExternalInput").ap() for l in range(3)]
    was_d = [nc.dram_tensor(f"was{l}", [32 if l == 0 else 8, 2], FP16, kind="ExternalInput").ap() for l in range(3)]
    wad_d = [nc.dram_tensor(f"wad{l}", [32 if l == 0 else 8, 1], FP16, kind="ExternalInput").ap() for l in range(3)]
    onescol_d = nc.dram_tensor("onescol", [1, PADF], FP16, kind="ExternalInput").ap()
    fc1A_d = nc.dram_tensor("fc1A", [8, HID], FP16, kind="ExternalInput").ap()
    b1c_d = nc.dram_tensor("b1c", [HID, 1], F32, kind="ExternalInput").ap()
    G40J_d = nc.dram_tensor("G40J", [40, 32, HID], FP16, kind="ExternalInput").ap()
    fc2J_d = nc.dram_tensor("fc2J", [HID, 32, 32], FP16, kind="ExternalInput").ap()
    b2col_d = nc.dram_tensor("b2col", [128, 1], F32, kind="ExternalInput").ap()

    out_d = nc.dram_tensor("out", [RPC, N], F32, kind="ExternalOutput").ap()
    encT_out_d = nc.dram_tensor("encT_out", [GOUT, PADF], FP16, kind="ExternalOutput").ap()

    with tile.TileContext(nc) as tc:
        _emit(tc, dict(
            featT_g=featT_g, featT_my=featT_my, comb=comb_d, mask=mask_d,
            dist=dist_d, W=W_d, was=was_d, wad=wad_d, onescol=onescol_d,
            fc1A=fc1A_d, b1c=b1c_d,
            G40J=G40J_d, fc2J=fc2J_d, b2col=b2col_d, out=out_d, encT_out=encT_out_d,
        ))
    nc.compile()
    return nc


def _emit(tc, io):
    nc = tc.nc
    from contextlib import ExitStack

    with ExitStack() as glob:
        pers = glob.enter_context(tc.tile_pool(name="pers", bufs=1))

        featT_sb = pers.tile([32, PADF], FP16)
        nc.sync.dma_start(out=featT_sb[:], in_=io["featT_g"][:])
        featT_my_sb = pers.tile([32, 256], FP16)
        nc.sync.dma_start(out=featT_my_sb[:], in_=io["featT_my"][:])
        comb_sb = pers.tile([128, 2, PADN], FP16)
        nc.sync.dma_start(out=comb_sb[:], in_=io["comb"].rearrange("(t p) n -> p t n", p=128))
        mask_sb = pers.tile([128, 2, PADN], FP16)
        nc.sync.dma_start(out=mask_sb[:], in_=io["mask"].rearrange("(t p) n -> p t n", p=128))

        W_sb, was_sb, wad_sb = [], [], []
        for l in range(3):
            k = 32 if l == 0 else 8
            w = pers.tile([k, 8], FP16, name=f"W{l}_sb")
            nc.sync.dma_start(out=w[:], in_=io["W"][l][:])
            W_sb.append(w)
            ws = pers.tile([k, 2], FP16, name=f"was{l}_sb")
            nc.sync.dma_start(out=ws[:], in_=io["was"][l][:])
            was_sb.append(ws)
            wd = pers.tile([k, 1], FP16, name=f"wad{l}_sb")
            nc.sync.dma_start(out=wd[:], in_=io["wad"][l][:])
            wad_sb.append(wd)

        fc1A_sb = pers.tile([8, HID], FP16)
        nc.sync.dma_start(out=fc1A_sb[:], in_=io["fc1A"][:])
        b1c_sb = pers.tile([HID, 1], F32)
        nc.sync.dma_start(out=b1c_sb[:], in_=io["b1c"][:])
        G40J_sb = pers.tile([40, 32, HID], FP16)
        nc.sync.dma_start(out=G40J_sb[:], in_=io["G40J"][:])
        fc2J_sb = pers.tile([HID, 32, 32], FP16)
        nc.sync.dma_start(out=fc2J_sb[:], in_=io["fc2J"][:])
        b2col_sb = pers.tile([128, 1], F32)
        nc.sync.dma_start(out=b2col_sb[:], in_=io["b2col"][:])

        ones_col = pers.tile([128, 1], FP16)
        nc.vector.memset(ones_col[:], 1.0)
        expb_col = pers.tile([128, 1], F32)
        nc.vector.memset(expb_col[:], EXP_BIAS)

        maskT_sb = pers.tile([128, 2, NBLK, 128], FP16)
        for t in range(2):
            for b in range(NBLK):
                nc.sync.dma_start_transpose(
                    maskT_sb[:, t, b, :], mask_sb[:, t, 128 * b:128 * b + 128])

        encT_full = pers.tile([GOUT, PADF], FP16)
        nc.vector.memset(encT_full[:, 1504:PADF], 0.0)
        encT_mine = pers.tile([GOUT, 256], FP16)
        nc.vector.memset(encT_mine[:, RPC:256], 0.0)
        h1aug = pers.tile([128, NBLK, 9], FP16)
        nc.sync.dma_start(out=h1aug[:, :, 8:9],
                          in_=io["onescol"][:, 0:PADN].rearrange("a (b p) -> p b a", p=128))
        fsd_g = pers.tile([2, PADF], FP16)
        nc.vector.memset(fsd_g[:], 1.0)
        fslT = pers.tile([2, 256], FP16)
        Hs = pers.tile([9, 1], F32)

        C_sb = pers.tile([HID, 256], F32)
        dstage = []
        for kb in range(2):
            ds = pers.tile([40, PADN], FP16, name=f"dstage_{kb}")
            nc.vector.memset(ds[:], 0.0)
            dstage.append(ds)

        dram = glob.enter_context(tc.tile_pool(name="dram", bufs=2, space="DRAM"))

        for l in range(3):
            K = 32 if l == 0 else 8
            src = featT_sb if l == 0 else encT_full
            src_my = featT_my_sb if l == 0 else encT_mine

            with ExitStack() as prep:
                pp = prep.enter_context(tc.tile_pool(name=f"prep{l}", bufs=1, space="PSUM"))
                ppsb = prep.enter_context(tc.tile_pool(name=f"prepsb{l}", bufs=2))

                for b in range(NBLK):
                    ph = pp.tile([128, 8], F32, tag="ph", bufs=2)
                    nc.tensor.matmul(ph[:], src[0:K, 128 * b:128 * b + 128],
                                     W_sb[l][0:K, :], start=True, stop=True)
                    nc.scalar.copy(h1aug[:, b, 0:8], ph[:])

                phs = pp.tile([9, 1], F32)
                for b in range(NBLK):
                    nc.tensor.matmul(phs[:], h1aug[:, b, :], ones_col[:],
                                     start=(b == 0), stop=(b == NBLK - 1))
                nc.scalar.mul(Hs[:], phs[:], EXP_SCALE)

                pfd = pp.tile([1, PADF], F32)
                for c0 in range(0, PADF, 512):
                    cw = min(512, PADF - c0)
                    nc.tensor.matmul(pfd[:, c0:c0 + cw], wad_sb[l][0:K, :],
                                     src[0:K, c0:c0 + cw], start=True, stop=True)
                nc.scalar.copy(fsd_g[0:1, :], pfd[:])

                pfs = pp.tile([2, 256], F32)
                nc.tensor.matmul(pfs[:], was_sb[l][0:K, :], src_my[0:K, :],
                                 start=True, stop=True)
                nc.scalar.copy(fslT[:], pfs[:])
                nc.vector.memset(fslT[0:1, :], 1.0)

            with ExitStack() as tp_:
                ps = tp_.enter_context(tc.tile_pool(name=f"gat{l}", bufs=2, space="PSUM"))
                sbp = tp_.enter_context(tc.tile_pool(name=f"gatsb{l}", bufs=2))
                for t in range(2):
                    nrow = 128 if t == 0 else RPC - 128

                    s_ps = ps.tile([128, PADN], F32, tag="s")
                    for c0 in range(0, PADN, 512):
                        nc.tensor.matmul(s_ps[:, c0:c0 + 512],
                                         fslT[0:2, 128 * t:128 * t + 128],
                                         fsd_g[0:2, c0:c0 + 512], start=True, stop=True)

                    r8 = sbp.tile([128, PADN], F32, tag="r8")
                    lr = sbp.tile([128, PADN], FP16, tag="lr")
                    m = sbp.tile([128, PADN], FP16, tag="m")
                    ee = sbp.tile([128, PADN], FP16, tag="ee")
                    for hh in range(2):
                        sl = slice(768 * hh, 768 * hh + 768)
                        nc.scalar.activation(r8[:, sl], s_ps[:, sl], AF.Relu, scale=0.8)
                        nc.vector.scalar_tensor_tensor(lr[:, sl], s_ps[:, sl], 0.2,
                                                       r8[:, sl], AL.mult, AL.add)
                        nc.vector.tensor_mul(m[:, sl], lr[:, sl], comb_sb[:, t, sl])
                        nc.scalar.activation(ee[:, sl], m[:, sl], AF.Exp, bias=expb_col[:])

                    pnz = ps.tile([9, 128], F32, tag="pnz", bufs=1, name="pnz")[:]
                    pmz = ps.tile([9, 128], F32, tag="pmz", bufs=1, name="pmz")[:]
                    for b in range(NBLK):
                        eeT = sbp.tile([128, 128], FP16, tag="eeT", bufs=6)
                        nc.sync.dma_start_transpose(eeT[:], ee[:, 128 * b:128 * b + 128])
                        nc.tensor.matmul(pnz, h1aug[:, b, :], eeT[:],
                                         start=(b == 0), stop=(b == NBLK - 1))
                        nc.tensor.matmul(pmz, h1aug[:, b, :], maskT_sb[:, t, b, :],
                                         start=(b == 0), stop=(b == NBLK - 1))

                    mzs = sbp.tile([9, 128], F32, tag="mzs")
                    nc.scalar.mul(mzs[:], pmz, EXP_SCALE)
                    num9 = sbp.tile([9, 128], F32, tag="num9")
                    nc.vector.scalar_tensor_tensor(num9[:], pnz, Hs[:], mzs[:],
                                                   AL.subtract, AL.add)
                    rz9 = sbp.tile([9, 128], F32, tag="rz9")
                    nc.vector.reciprocal(rz9[:], num9[:])
                    rzs = sbp.tile([1, 128], F32, tag="rzs")
                    nc.sync.dma_start(out=rzs[:], in_=rz9[8:9, :])
                    rzb = sbp.tile([8, 128], F32, tag="rzb")
                    nc.gpsimd.partition_broadcast(rzb[:], rzs[:])

                    epre = sbp.tile([8, 128], F32, tag="epre")
                    nc.vector.tensor_mul(epre[:], num9[0:8, :], rzb[:])
                    xm = sbp.tile([8, 128], F32, tag="xm")
                    nc.vector.tensor_scalar_min(xm[:], epre[:], 0.0)
                    ex = sbp.tile([8, 128], F32, tag="ex")
                    nc.scalar.activation(ex[:], xm[:], AF.Exp)
                    en = sbp.tile([8, 128], F32, tag="en")
                    nc.vector.scalar_tensor_tensor(en[:], epre[:], 0.0, ex[:],
                                                   AL.max, AL.add)
                    nc.vector.tensor_scalar_sub(
                        encT_mine[:, 128 * t:128 * t + nrow], en[:, 0:nrow], 1.0)

            ib = dram.tile([GOUT, RPC], FP16, tag="ag_in")
            ob = dram.tile([NC, GOUT, RPC], FP16, tag="ag_out")
            nc.gpsimd.dma_start(out=ib[:], in_=encT_mine[:, 0:RPC])
            nc.gpsimd.collective_compute(
                "AllGather", AL.bypass, replica_groups=[list(range(NC))],
                ins=[ib.opt()], outs=[ob.opt()])
            nc.gpsimd.dma_start(
                out=encT_full[:, 0:NC * RPC].rearrange("p (c r) -> p c r", c=NC),
                in_=ob.rearrange("c p r -> p c r"))

        nc.sync.dma_start(out=io["encT_out"][:], in_=encT_full[:])

        with ExitStack() as cprep:
            pc = cprep.enter_context(tc.tile_pool(name="pc", bufs=1, space="PSUM"))
            c_ps = pc.tile([128, 256], F32)
            nc.tensor.matmul(c_ps[:], fc1A_sb[:], encT_mine[0:8, :],
                             start=True, stop=True)
            nc.scalar.activation(C_sb[:], c_ps[:], AF.Identity, bias=b1c_sb[:])

        with ExitStack() as mlp:
            psh = mlp.enter_context(tc.tile_pool(name="psh", bufs=4, space="PSUM"))
            pso = mlp.enter_context(tc.tile_pool(name="pso", bufs=1, space="PSUM"))
            msb = mlp.enter_context(tc.tile_pool(name="msb", bufs=3))
            osb = mlp.enter_context(tc.tile_pool(name="osb", bufs=2))

            for ds in dstage:
                nc.vector.tensor_copy(ds[32:40, :], encT_full[0:8, 0:PADN])

            nblocks = [(0, 128), (128, RPC - 128)]
            for blk0, brows in nblocks:
                po = [pso.tile([128, 512], F32, tag=f"po{k}", name=f"po{k}_{blk0}")
                      for k in range(3)]
                orow_big = osb.tile([128, PADN], F32, tag="orow")
                for ii in range(brows):
                    i = blk0 + ii
                    j32 = i % 32
                    ds = dstage[(i // 32) % 2]
                    if j32 == 0:
                        nb = min(32, RPC - i)
                        nc.sync.dma_start(out=ds[0:nb, 0:N],
                                          in_=io["dist"][i:i + nb, :])

                    hid = msb.tile([128, PADN], FP16, tag="hid")
                    ci = C_sb[:, i:i + 1]
                    for k3 in range(3):
                        ph = psh.tile([128, 512], F32, tag="ph", bufs=4)
                        c0 = 512 * k3
                        nc.tensor.matmul(ph[:], G40J_sb[:, j32, :],
                                         ds[0:40, c0:c0 + 512],
                                         start=True, stop=True)
                        if (i + k3) % 2 == 0:
                            nc.vector.tensor_scalar(hid[:, c0:c0 + 512], ph[:], ci,
                                                    0.0, AL.add, AL.max)
                        else:
                            nc.scalar.activation(hid[:, c0:c0 + 512], ph[:], AF.Relu,
                                                 bias=ci)

                    c, j = ii // 32, ii % 32
                    for k in range(3):
                        nc.tensor.matmul(
                            po[k][32 * c:32 * c + 32, :], fc2J_sb[:, j, :],
                            hid[:, 512 * k:512 * k + 512],
                            start=(j == 0), stop=(j == 31 or ii == brows - 1),
                            tile_position=(0, 32 * c))

                prows = ((brows + 31) // 32) * 32
                for k in range(3):
                    if (k % 2) == 0:
                        nc.vector.tensor_scalar_add(
                            orow_big[0:prows, 512 * k:512 * k + 512],
                            po[k][0:prows, :], b2col_sb[0:prows, :])
                    else:
                        nc.scalar.activation(orow_big[0:prows, 512 * k:512 * k + 512],
                                             po[k][0:prows, :], AF.Identity,
                                             bias=b2col_sb[0:prows, :])
                nc.sync.dma_start(out=io["out"][blk0:blk0 + brows, :],
                                  in_=orow_big[0:brows, 0:N])



def _prep_inputs(geo_adj, sem_adj, features, distance_rows,
                 W0, W1, W2, a0, a1, a2, fc1_w, fc1_b, fc2_w, fc2_b):
    f32 = np.float32
    fp16 = np.float16
    comb = (geo_adj + sem_adj).astype(fp16)
    mask01 = (comb > 0).astype(fp16)

    featT_g = np.zeros((32, PADF), fp16)
    featT_g[:, :N] = features.T

    Ws = [W0.astype(fp16), W1.astype(fp16), W2.astype(fp16)]
    aas = [a0.astype(f32), a1.astype(f32), a2.astype(f32)]

    was, wad = [], []
    for W, a in zip(Ws, aas):
        s = (W.astype(f32) @ a[:GOUT]).astype(fp16)
        d = (W.astype(f32) @ a[GOUT:]).astype(fp16)
        ws = np.zeros((W.shape[0], 2), fp16)
        ws[:, 1:2] = s
        was.append(ws)
        wad.append(d)

    onescol = np.zeros((1, PADF), fp16)
    onescol[0, :N] = 1.0

    fc1A = fc1_w[0:8].astype(fp16)
    b1c = fc1_b.reshape(HID, 1).astype(f32)
    G40J = np.zeros((40, 32, HID), fp16)
    for j in range(32):
        G40J[j, j, :] = fc1_w[16]
        G40J[32:40, j, :] = fc1_w[8:16]

    fc2J = np.zeros((HID, 32, 32), fp16)
    for j in range(32):
        fc2J[:, j, j] = fc2_w.reshape(HID)
    b2col = np.full((128, 1), float(np.asarray(fc2_b).reshape(())), f32)

    in_maps = []
    for c in range(NC):
        rows = np.clip(np.arange(c * RPC, c * RPC + 256), 0, N - 1)
        comb_c = np.zeros((256, PADN), fp16)
        comb_c[:, :N] = comb[rows]
        mask_c = np.zeros((256, PADN), fp16)
        mask_c[:, :N] = mask01[rows]
        dist_c = distance_rows[np.clip(np.arange(c * RPC, c * RPC + RPC), 0, N - 1)].astype(fp16)
        featT_my = np.zeros((32, 256), fp16)
        featT_my[:, :] = features.T[:, rows]
        m = {
            "featT_g": featT_g, "featT_my": featT_my,
            "comb": comb_c, "mask": mask_c, "dist": dist_c, "onescol": onescol,
            "fc1A": fc1A, "b1c": b1c, "G40J": G40J, "fc2J": fc2J, "b2col": b2col,
        }
        for l in range(3):
            m[f"W{l}"] = Ws[l]
            m[f"was{l}"] = was[l]
            m[f"wad{l}"] = wad[l]
        in_maps.append(m)
    return in_maps


def _is_meshgrid(region_pairs):
    rp = np.asarray(region_pairs)
    if rp.shape != (N * N, 2):
        return False
    k = np.arange(N * N, dtype=np.int64)
    return bool(np.array_equal(rp[:, 0], k // N) and np.array_equal(rp[:, 1], k % N))


def _host_mlp(enc, region_pairs, distance_features, fc1_w, fc1_b, fc2_w, fc2_b):
    rp = np.asarray(region_pairs).astype(np.int64)
    n = rp.shape[0]
    out = np.empty((n, 1), np.float32)
    A = fc1_w[0:8].astype(np.float32)
    B = fc1_w[8:16].astype(np.float32)
    w16 = fc1_w[16:17].astype(np.float32)
    u = enc @ A
    v = enc @ B
    for s in range(0, n, 262144):
        e = min(s + 262144, n)
        h = u[rp[s:e, 0]] + v[rp[s:e, 1]] + distance_features[s:e].astype(np.float32) @ w16 + fc1_b
        np.maximum(h, 0, out=h)
        out[s:e] = h @ fc2_w + fc2_b
    return out


def kernel(**inputs):
    geo_adj = np.asarray(inputs["geo_adj"], np.float32)
    sem_adj = np.asarray(inputs["sem_adj"], np.float32)
    features = np.asarray(inputs["features"], np.float32)
    region_pairs = inputs["region_pairs"]
    distance_features = np.asarray(inputs["distance_features"], np.float32)
    fc1_w = np.asarray(inputs["fc1_w"], np.float32)
    fc1_b = np.asarray(inputs["fc1_b"], np.float32)
    fc2_w = np.asarray(inputs["fc2_w"], np.float32)
    fc2_b = np.asarray(inputs["fc2_b"], np.float32)

    mesh = _is_meshgrid(region_pairs)
    if mesh:
        dist_rows = distance_features.reshape(N, N)
    else:
        dist_rows = np.zeros((N, N), np.float32)

    in_maps = _prep_inputs(
        geo_adj, sem_adj, features, dist_rows,
        inputs["W0"], inputs["W1"], inputs["W2"],
        inputs["a0"], inputs["a1"], inputs["a2"],
        fc1_w, fc1_b, fc2_w, fc2_b)

    if "nc" not in _CACHED:
        _CACHED["nc"] = _build_nc()
    nc = _CACHED["nc"]

    res = run_bass_kernel_spmd(nc, in_maps, core_ids=list(range(NC)))

    if mesh:
        rows = np.concatenate([res.results[c]["out"] for c in range(NC)], axis=0)
        out = rows[:N].reshape(N * N, 1).astype(np.float32)
    else:
        encT = res.results[0]["encT_out"][:, :N]
        out = _host_mlp(encT.T.astype(np.float32), region_pairs,
                        distance_features, fc1_w, fc1_b, fc2_w, fc2_b)
    return out


# revision 57
# speedup vs baseline: 3001.6335x; 1.0166x over previous
import sys

for p in ("/opt/trn_rl_repo", "/opt/trn_rl_repo/concourse"):
    if p not in sys.path:
        sys.path.insert(0, p)

import numpy as np
import ml_dtypes

import concourse.bass as bass
import concourse.bacc as bacc
import concourse.tile as tile
from concourse import mybir
from concourse.bass_utils import run_bass_kernel_spmd

F32 = mybir.dt.float32
BF16 = mybir.dt.bfloat16
FP16 = mybir.dt.float16
FP8 = mybir.dt.float8e4
AL = mybir.AluOpType
AF = mybir.ActivationFunctionType

EXP_SHIFT_K = 6
EXP_BIAS = float(-EXP_SHIFT_K * np.log(2.0))
EXP_SCALE = float(2.0 ** -EXP_SHIFT_K)

N = 1500
NC = 8
RPC = 188
PADN = 1536
PADF = 1664
GOUT = 8
HID = 128
NBLK = 12

_CACHED = {}


def _build_nc():
    nc = bacc.Bacc("TRN2", target_bir_lowering=False, debug=False, num_devices=NC)

    featT_g = nc.dram_tensor("featT_g", [32, PADF], FP16, kind="ExternalInput").ap()
    featT_my = nc.dram_tensor("featT_my", [32, 256], FP16, kind="ExternalInput").ap()
    comb_d = nc.dram_tensor("comb", [256, PADN], FP16, kind="ExternalInput").ap()
    mask_d = nc.dram_tensor("mask", [256, PADN], FP16, kind="ExternalInput").ap()
    dist_d = nc.dram_tensor("dist", [RPC, N], FP16, kind="ExternalInput").ap()
    W_d = [nc.dram_tensor(f"W{l}", [32 if l == 0 else 8, 8], FP16, kind="ExternalInput").ap() for l in range(3)]
    was_d = [nc.dram_tensor(f"was{l}", [32 if l == 0 else 8, 2], FP16, kind="ExternalInput").ap() for l in range(3)]
    wad_d = [nc.dram_tensor(f"wad{l}", [32 if l == 0 else 8, 1], FP16, kind="ExternalInput").ap() for l in range(3)]
    onescol_d = nc.dram_tensor("onescol", [1, PADF], FP16, kind="ExternalInput").ap()
    fc1A_d = nc.dram_tensor("fc1A", [8, HID], FP16, kind="ExternalInput").ap()
    b1c_d = nc.dram_tensor("b1c", [HID, 1], F32, kind="ExternalInput").ap()
    G40J_d = nc.dram_tensor("G40J", [40, 32, HID], FP16, kind="ExternalInput").ap()
    fc2J_d = nc.dram_tensor("fc2J", [HID, 32, 32], FP16, kind="ExternalInput").ap()
    b2col_d = nc.dram_tensor("b2col", [128, 1], F32, kind="ExternalInput").ap()

    out_d = nc.dram_tensor("out", [RPC, N], F32, kind="ExternalOutput").ap()
    encT_out_d = nc.dram_tensor("encT_out", [GOUT, PADF], FP16, kind="ExternalOutput").ap()

    with tile.TileContext(nc) as tc:
        _emit(tc, dict(
            featT_g=featT_g, featT_my=featT_my, comb=comb_d, mask=mask_d,
            dist=dist_d, W=W_d, was=was_d, wad=wad_d, onescol=onescol_d,
            fc1A=fc1A_d, b1c=b1c_d,
            G40J=G40J_d, fc2J=fc2J_d, b2col=b2col_d, out=out_d, encT_out=encT_out_d,
        ))
    nc.compile()
    return nc


def _emit(tc, io):
    nc = tc.nc
    from contextlib import ExitStack

    with ExitStack() as glob:
        pers = glob.enter_context(tc.tile_pool(name="pers", bufs=1))

        featT_sb = pers.tile([32, PADF], FP16)
        nc.sync.dma_start(out=featT_sb[:], in_=io["featT_g"][:])
        featT_my_sb = pers.tile([32, 256], FP16)
        nc.sync.dma_start(out=featT_my_sb[:], in_=io["featT_my"][:])
        comb_sb = pers.tile([128, 2, PADN], FP16)
        nc.sync.dma_start(out=comb_sb[:], in_=io["comb"].rearrange("(t p) n -> p t n", p=128))
        mask_sb = pers.tile([128, 2, PADN], FP16)
        nc.sync.dma_start(out=mask_sb[:], in_=io["mask"].rearrange("(t p) n -> p t n", p=128))

        W_sb, was_sb, wad_sb = [], [], []
        for l in range(3):
            k = 32 if l == 0 else 8
            w = pers.tile([k, 8], FP16, name=f"W{l}_sb")
            nc.sync.dma_start(out=w[:], in_=io["W"][l][:])
            W_sb.append(w)
            ws = pers.tile([k, 2], FP16, name=f"was{l}_sb")
            nc.sync.dma_start(out=ws[:], in_=io["was"][l][:])
            was_sb.append(ws)
            wd = pers.tile([k, 1], FP16, name=f"wad{l}_sb")
            nc.sync.dma_start(out=wd[:], in_=io["wad"][l][:])
            wad_sb.append(wd)

        fc1A_sb = pers.tile([8, HID], FP16)
        nc.sync.dma_start(out=fc1A_sb[:], in_=io["fc1A"][:])
        b1c_sb = pers.tile([HID, 1], F32)
        nc.sync.dma_start(out=b1c_sb[:], in_=io["b1c"][:])
        G40J_sb = pers.tile([40, 32, HID], FP16)
        nc.sync.dma_start(out=G40J_sb[:], in_=io["G40J"][:])
        fc2J_sb = pers.tile([HID, 32, 32], FP16)
        nc.sync.dma_start(out=fc2J_sb[:], in_=io["fc2J"][:])
        b2col_sb = pers.tile([128, 1], F32)
        nc.sync.dma_start(out=b2col_sb[:], in_=io["b2col"][:])

        ones_col = pers.tile([128, 1], FP16)
        nc.vector.memset(ones_col[:], 1.0)
        expb_col = pers.tile([128, 1], F32)
        nc.vector.memset(expb_col[:], EXP_BIAS)

        maskT_sb = pers.tile([128, 2, NBLK, 128], FP16)
        for t in range(2):
            for b in range(NBLK):
                nc.sync.dma_start_transpose(
                    maskT_sb[:, t, b, :], mask_sb[:, t, 128 * b:128 * b + 128])

        encT_full = pers.tile([GOUT, PADF], FP16)
        nc.vector.memset(encT_full[:, 1504:PADF], 0.0)
        encT_mine = pers.tile([GOUT, 256], FP16)
        nc.vector.memset(encT_mine[:, RPC:256], 0.0)
        h1aug = pers.tile([128, NBLK, 9], FP16)
        nc.sync.dma_start(out=h1aug[:, :, 8:9],
                          in_=io["onescol"][:, 0:PADN].rearrange("a (b p) -> p b a", p=128))
        fsd_g = pers.tile([2, PADF], FP16)
        nc.vector.memset(fsd_g[:], 1.0)
        fslT = pers.tile([2, 256], FP16)
        Hs = pers.tile([9, 1], F32)

        C_sb = pers.tile([HID, 256], F32)
        dstage = []
        for kb in range(2):
            ds = pers.tile([40, PADN], FP16, name=f"dstage_{kb}")
            nc.vector.memset(ds[:], 0.0)
            dstage.append(ds)

        dram = glob.enter_context(tc.tile_pool(name="dram", bufs=2, space="DRAM"))

        for l in range(3):
            K = 32 if l == 0 else 8
            src = featT_sb if l == 0 else encT_full
            src_my = featT_my_sb if l == 0 else encT_mine

            with ExitStack() as prep:
                pp = prep.enter_context(tc.tile_pool(name=f"prep{l}", bufs=1, space="PSUM"))
                ppsb = prep.enter_context(tc.tile_pool(name=f"prepsb{l}", bufs=2))

                for b in range(NBLK):
                    ph = pp.tile([128, 8], F32, tag="ph", bufs=2)
                    nc.tensor.matmul(ph[:], src[0:K, 128 * b:128 * b + 128],
                                     W_sb[l][0:K, :], start=True, stop=True)
                    nc.scalar.copy(h1aug[:, b, 0:8], ph[:])

                phs = pp.tile([9, 1], F32)
                for b in range(NBLK):
                    nc.tensor.matmul(phs[:], h1aug[:, b, :], ones_col[:],
                                     start=(b == 0), stop=(b == NBLK - 1))
                nc.scalar.mul(Hs[:], phs[:], EXP_SCALE)

                pfd = pp.tile([1, PADF], F32)
                for c0 in range(0, PADF, 512):
                    cw = min(512, PADF - c0)
                    nc.tensor.matmul(pfd[:, c0:c0 + cw], wad_sb[l][0:K, :],
                                     src[0:K, c0:c0 + cw], start=True, stop=True)
                    nc.scalar.copy(fsd_g[0:1, c0:c0 + cw], pfd[:, c0:c0 + cw])

                pfs = pp.tile([2, 256], F32)
                nc.tensor.matmul(pfs[:], was_sb[l][0:K, :], src_my[0:K, :],
                                 start=True, stop=True)
                nc.scalar.copy(fslT[:], pfs[:])
                nc.vector.memset(fslT[0:1, :], 1.0)

            with ExitStack() as tp_:
                ps = tp_.enter_context(tc.tile_pool(name=f"gat{l}", bufs=2, space="PSUM"))
                sbp = tp_.enter_context(tc.tile_pool(name=f"gatsb{l}", bufs=2))
                for t in range(2):
                    nrow = 128 if t == 0 else RPC - 128

                    s_ps = ps.tile([128, PADN], F32, tag="s")
                    for c0 in range(0, PADN, 512):
                        nc.tensor.matmul(s_ps[:, c0:c0 + 512],
                                         fslT[0:2, 128 * t:128 * t + 128],
                                         fsd_g[0:2, c0:c0 + 512], start=True, stop=True)

                    r8 = sbp.tile([128, PADN], F32, tag="r8")
                    nc.scalar.activation(r8[:], s_ps[:], AF.Relu, scale=0.8)
                    lr = sbp.tile([128, PADN], FP16, tag="lr")
                    nc.vector.scalar_tensor_tensor(lr[:], s_ps[:], 0.2, r8[:],
                                                   AL.mult, AL.add)
                    m = sbp.tile([128, PADN], FP16, tag="m")
                    nc.vector.tensor_mul(m[:], lr[:], comb_sb[:, t, :])
                    ee = sbp.tile([128, PADN], FP16, tag="ee")
                    nc.scalar.activation(ee[:], m[:], AF.Exp, bias=expb_col[:])

                    pnz = ps.tile([9, 128], F32, tag="pnz", bufs=1, name="pnz")[:]
                    pmz = ps.tile([9, 128], F32, tag="pmz", bufs=1, name="pmz")[:]
                    for b in range(NBLK):
                        eeT = sbp.tile([128, 128], FP16, tag="eeT", bufs=6)
                        nc.sync.dma_start_transpose(eeT[:], ee[:, 128 * b:128 * b + 128])
                        nc.tensor.matmul(pnz, h1aug[:, b, :], eeT[:],
                                         start=(b == 0), stop=(b == NBLK - 1))
                        nc.tensor.matmul(pmz, h1aug[:, b, :], maskT_sb[:, t, b, :],
                                         start=(b == 0), stop=(b == NBLK - 1))

                    mzs = sbp.tile([9, 128], F32, tag="mzs")
                    nc.scalar.mul(mzs[:], pmz, EXP_SCALE)
                    num9 = sbp.tile([9, 128], F32, tag="num9")
                    nc.vector.scalar_tensor_tensor(num9[:], pnz, Hs[:], mzs[:],
                                                   AL.subtract, AL.add)
                    rz9 = sbp.tile([9, 128], F32, tag="rz9")
                    nc.vector.reciprocal(rz9[:], num9[:])
                    rzs = sbp.tile([1, 128], F32, tag="rzs")
                    nc.sync.dma_start(out=rzs[:], in_=rz9[8:9, :])
                    rzb = sbp.tile([8, 128], F32, tag="rzb")
                    nc.gpsimd.partition_broadcast(rzb[:], rzs[:])

                    epre = sbp.tile([8, 128], F32, tag="epre")
                    nc.vector.tensor_mul(epre[:], num9[0:8, :], rzb[:])
                    xm = sbp.tile([8, 128], F32, tag="xm")
                    nc.vector.tensor_scalar_min(xm[:], epre[:], 0.0)
                    ex = sbp.tile([8, 128], F32, tag="ex")
                    nc.scalar.activation(ex[:], xm[:], AF.Exp)
                    en = sbp.tile([8, 128], F32, tag="en")
                    nc.vector.scalar_tensor_tensor(en[:], epre[:], 0.0, ex[:],
                                                   AL.max, AL.add)
                    nc.vector.tensor_scalar_sub(
                        encT_mine[:, 128 * t:128 * t + nrow], en[:, 0:nrow], 1.0)

            ib = dram.tile([GOUT, RPC], FP16, tag="ag_in")
            ob = dram.tile([NC, GOUT, RPC], FP16, tag="ag_out")
            nc.gpsimd.dma_start(out=ib[:], in_=encT_mine[:, 0:RPC])
            nc.gpsimd.collective_compute(
                "AllGather", AL.bypass, replica_groups=[list(range(NC))],
                ins=[ib.opt()], outs=[ob.opt()])
            nc.gpsimd.dma_start(
                out=encT_full[:, 0:NC * RPC].rearrange("p (c r) -> p c r", c=NC),
                in_=ob.rearrange("c p r -> p c r"))

        nc.sync.dma_start(out=io["encT_out"][:], in_=encT_full[:])

        with ExitStack() as cprep:
            pc = cprep.enter_context(tc.tile_pool(name="pc", bufs=1, space="PSUM"))
            c_ps = pc.tile([128, 256], F32)
            nc.tensor.matmul(c_ps[:], fc1A_sb[:], encT_mine[0:8, :],
                             start=True, stop=True)
            nc.scalar.activation(C_sb[:], c_ps[:], AF.Identity, bias=b1c_sb[:])

        with ExitStack() as mlp:
            psh = mlp.enter_context(tc.tile_pool(name="psh", bufs=4, space="PSUM"))
            pso = mlp.enter_context(tc.tile_pool(name="pso", bufs=1, space="PSUM"))
            msb = mlp.enter_context(tc.tile_pool(name="msb", bufs=3))
            osb = mlp.enter_context(tc.tile_pool(name="osb", bufs=2))

            for ds in dstage:
                nc.vector.tensor_copy(ds[32:40, :], encT_full[0:8, 0:PADN])

            nblocks = [(0, 128), (128, RPC - 128)]
            for blk0, brows in nblocks:
                po = [pso.tile([128, 512], F32, tag=f"po{k}", name=f"po{k}_{blk0}")
                      for k in range(3)]
                orow_big = osb.tile([128, PADN], F32, tag="orow")
                for ii in range(brows):
                    i = blk0 + ii
                    j32 = i % 32
                    ds = dstage[(i // 32) % 2]
                    if j32 == 0:
                        nb = min(32, RPC - i)
                        nc.sync.dma_start(out=ds[0:nb, 0:N],
                                          in_=io["dist"][i:i + nb, :])

                    hid = msb.tile([128, PADN], FP16, tag="hid")
                    ci = C_sb[:, i:i + 1]
                    for k3 in range(3):
                        cw = 512 if k3 < 2 else N - 1024
                        ph = psh.tile([128, 512], F32, tag="ph", bufs=4)
                        c0 = 512 * k3
                        nc.tensor.matmul(ph[:, 0:cw], G40J_sb[:, j32, :],
                                         ds[0:40, c0:c0 + cw],
                                         start=True, stop=True)
                        if (i + k3) % 2 == 0:
                            nc.vector.tensor_scalar(hid[:, c0:c0 + cw], ph[:, 0:cw],
                                                    ci, 0.0, AL.add, AL.max)
                        else:
                            nc.scalar.activation(hid[:, c0:c0 + cw], ph[:, 0:cw],
                                                 AF.Relu, bias=ci)

                    c, j = ii // 32, ii % 32
                    for k in range(3):
                        cw = 512 if k < 2 else N - 1024
                        nc.tensor.matmul(
                            po[k][32 * c:32 * c + 32, 0:cw], fc2J_sb[:, j, :],
                            hid[:, 512 * k:512 * k + cw],
                            start=(j == 0), stop=(j == 31 or ii == brows - 1),
                            tile_position=(0, 32 * c))

                prows = ((brows + 31) // 32) * 32
                for k in range(3):
                    cw = 512 if k < 2 else N - 1024
                    if (k % 2) == 0:
                        nc.vector.tensor_scalar_add(
                            orow_big[0:prows, 512 * k:512 * k + cw],
                            po[k][0:prows, 0:cw], b2col_sb[0:prows, :])
                    else:
                        nc.scalar.activation(orow_big[0:prows, 512 * k:512 * k + cw],
                                             po[k][0:prows, 0:cw], AF.Identity,
                                             bias=b2col_sb[0:prows, :])
                nc.sync.dma_start(out=io["out"][blk0:blk0 + brows, :],
                                  in_=orow_big[0:brows, 0:N])



def _prep_inputs(geo_adj, sem_adj, features, distance_rows,
                 W0, W1, W2, a0, a1, a2, fc1_w, fc1_b, fc2_w, fc2_b):
    f32 = np.float32
    fp16 = np.float16
    comb = (geo_adj + sem_adj).astype(fp16)
    mask01 = (comb > 0).astype(fp16)

    featT_g = np.zeros((32, PADF), fp16)
    featT_g[:, :N] = features.T

    Ws = [W0.astype(fp16), W1.astype(fp16), W2.astype(fp16)]
    aas = [a0.astype(f32), a1.astype(f32), a2.astype(f32)]

    was, wad = [], []
    for W, a in zip(Ws, aas):
        s = (W.astype(f32) @ a[:GOUT]).astype(fp16)
        d = (W.astype(f32) @ a[GOUT:]).astype(fp16)
        ws = np.zeros((W.shape[0], 2), fp16)
        ws[:, 1:2] = s
        was.append(ws)
        wad.append(d)

    onescol = np.zeros((1, PADF), fp16)
    onescol[0, :N] = 1.0

    fc1A = fc1_w[0:8].astype(fp16)
    b1c = fc1_b.reshape(HID, 1).astype(f32)
    G40J = np.zeros((40, 32, HID), fp16)
    for j in range(32):
        G40J[j, j, :] = fc1_w[16]
        G40J[32:40, j, :] = fc1_w[8:16]

    fc2J = np.zeros((HID, 32, 32), fp16)
    for j in range(32):
        fc2J[:, j, j] = fc2_w.reshape(HID)
    b2col = np.full((128, 1), float(np.asarray(fc2_b).reshape(())), f32)

    in_maps = []
    for c in range(NC):
        rows = np.clip(np.arange(c * RPC, c * RPC + 256), 0, N - 1)
        comb_c = np.zeros((256, PADN), fp16)
        comb_c[:, :N] = comb[rows]
        mask_c = np.zeros((256, PADN), fp16)
        mask_c[:, :N] = mask01[rows]
        dist_c = distance_rows[np.clip(np.arange(c * RPC, c * RPC + RPC), 0, N - 1)].astype(fp16)
        featT_my = np.zeros((32, 256), fp16)
        featT_my[:, :] = features.T[:, rows]
        m = {
            "featT_g": featT_g, "featT_my": featT_my,
            "comb": comb_c, "mask": mask_c, "dist": dist_c, "onescol": onescol,
            "fc1A": fc1A, "b1c": b1c, "G40J": G40J, "fc2J": fc2J, "b2col": b2col,
        }
        for l in range(3):
            m[f"W{l}"] = Ws[l]
            m[f"was{l}"] = was[l]
            m[f"wad{l}"] = wad[l]
        in_maps.append(m)
    return in_maps


def _is_meshgrid(region_pairs):
    rp = np.asarray(region_pairs)
    if rp.shape != (N * N, 2):
        return False
    k = np.arange(N * N, dtype=np.int64)
    return bool(np.array_equal(rp[:, 0], k // N) and np.array_equal(rp[:, 1], k % N))


def _host_mlp(enc, region_pairs, distance_features, fc1_w, fc1_b, fc2_w, fc2_b):
    rp = np.asarray(region_pairs).astype(np.int64)
    n = rp.shape[0]
    out = np.empty((n, 1), np.float32)
    A = fc1_w[0:8].astype(np.float32)
    B = fc1_w[8:16].astype(np.float32)
    w16 = fc1_w[16:17].astype(np.float32)
    u = enc @ A
    v = enc @ B
    for s in range(0, n, 262144):
        e = min(s + 262144, n)
        h = u[rp[s:e, 0]] + v[rp[s:e, 1]] + distance_features[s:e].astype(np.float32) @ w16 + fc1_b
        np.maximum(h, 0, out=h)
        out[s:e] = h @ fc2_w + fc2_b
    return out


def kernel(**inputs):
    geo_adj = np.asarray(inputs["geo_adj"], np.float32)
    sem_adj = np.asarray(inputs["sem_adj"], np.float32)
    features = np.asarray(inputs["features"], np.float32)
    region_pairs = inputs["region_pairs"]
    distance_features = np.asarray(inputs["distance_features"], np.float32)
    fc1_w = np.asarray(inputs["fc1_w"], np.float32)
    fc1_b = np.asarray(inputs["fc1_b"], np.float32)
    fc2_w = np.asarray(inputs["fc2_w"], np.float32)
    fc2_b = np.asarray(inputs["fc2_b"], np.float32)

    mesh = _is_meshgrid(region_pairs)
    if mesh:
        dist_rows = distance_features.reshape(N, N)
    else:
        dist_rows = np.zeros((N, N), np.float32)

    in_maps = _prep_inputs(
        geo_adj, sem_adj, features, dist_rows,
        inputs["W0"], inputs["W1"], inputs["W2"],
        inputs["a0"], inputs["a1"], inputs["a2"],
        fc1_w, fc1_b, fc2_w, fc2_b)

    if "nc" not in _CACHED:
        _CACHED["nc"] = _build_nc()
    nc = _CACHED["nc"]

    res = run_bass_kernel_spmd(nc, in_maps, core_ids=list(range(NC)))

    if mesh:
        rows = np.concatenate([res.results[c]["out"] for c in range(NC)], axis=0)
        out = rows[:N].reshape(N * N, 1).astype(np.float32)
    else:
        encT = res.results[0]["encT_out"][:, :N]
        out = _host_mlp(encT.T.astype(np.float32), region_pairs,
                        distance_features, fc1_w, fc1_b, fc2_w, fc2_b)
    return out
